# revision 23
# baseline (speedup 1.0000x reference)
"""Trainium2 Bass kernel for GQA attention prefill (B=2,T=2048,D=4096,H=32,KVH=8).

Sharding: data-parallel over batch (2) x tensor-parallel over heads (4 groups
of 8 q-heads / 2 kv-heads). 8 cores. Each core emits TWO partial o_proj
outputs (head-halves 0-3 / 4-7); host sums the 8 partials per batch.

v2 design (vs v1 baseline 1.03ms):
  - q/k projections in fp8 (e4m3) with DoubleRow matmuls (256-deep
    contraction per pass). x scaled by 16, wq/wk by 64; the 2^20 factor on
    scores is folded into the exp() activation scale.
  - v projection x-stationary -> vS produced directly in [tok, hd] layout
    (no PE transpose pass).
  - softmax denominator via DVE add-chain over exp tiles + gpsimd
    partition-reduce + fast reciprocal (replaces 109us of ones-matmuls).
  - o_proj split by head-halves; its matmuls are issued interleaved with
    the second attention window so the PE array stays busy while Scalar
    does exp(); remaining windows interleave q-proj with attention.

Per-core layouts:
  xq8D  [4,16,128,1024] fp8  x8[tb,s,p,j*512+n]  = 16*x[b, 512tb+n, 256s+128j+p]
  wq8D  [16,128,2048]   fp8  [s,p,j*1024+m]      = 64*wq_perm[m, 256s+128j+p]
  wk8D  [16,128,512]    fp8  [s,p,j*256+m]       = 64*wk_perm[m, ...]
  xbfD  [4096,2048]     bf16 x[b].T
  wvTD  [4096,256]      bf16
  woTD  [1024,4096]     bf16
  cosC/sinS [128,2048]  bf16 rope tables (rows 0:64 even dims, 64:128 odd;
                             sinS rows 0:64 negated)
"""

import numpy as np
import ml_dtypes

import concourse.bass as bass
import concourse.tile as tile
from concourse import bacc, mybir
from concourse.bass_isa import ReduceOp
from concourse.bass_utils import run_bass_kernel_spmd

BF16 = mybir.dt.bfloat16
F32 = mybir.dt.float32
FP8 = mybir.dt.float8e4
BT, T, D = 2, 2048, 4096
H, KVH, HD = 32, 8, 128
NQ, NKV = 8, 2          # per-core q heads / kv heads
NG = 4                  # head groups
SCALE = 1.0 / np.sqrt(128.0)
XS, WS = 16.0, 64.0     # fp8 scale factors for x and wq/wk
ESCALE = float(SCALE / (XS * XS * WS * WS))
DR = mybir.MatmulPerfMode.DoubleRow

_CACHE = {}


def _rope_evac(nc, sb, ps, out_sl, c_sl, s_sl):
    """ps: PSUM [128,512] f32 -> out_sl: SBUF bf16 [128,512] with RoPE.
    Rows 0:64 = even dims, 64:128 = odd dims (host-permuted weights).
    out = ps*C + shift64(ps)*S, via partition-shifted DVE reads."""
    tmp = sb.tile([128, 512], F32, tag="rtmp", name="rtmp")
    nc.vector.tensor_mul(tmp[0:64, :], ps[64:128, :], s_sl[0:64, :])
    nc.vector.tensor_mul(tmp[64:128, :], ps[0:64, :], s_sl[64:128, :])
    tmp2 = sb.tile([128, 512], F32, tag="rtmp2", name="rtmp2")
    nc.vector.tensor_mul(tmp2[:], ps[:], c_sl)
    nc.vector.tensor_add(out_sl, tmp2[:], tmp[:])


def _build():
    if "nc" in _CACHE:
        return _CACHE["nc"]
    nc = bacc.Bacc("TRN2", target_bir_lowering=False, debug=False, num_devices=8)
    xq8D = nc.dram_tensor("xq8", [4, 8, 128, 2048], FP8, kind="ExternalInput").ap()
    wq8D = nc.dram_tensor("wq8", [128, 16, 2048], FP8, kind="ExternalInput").ap()
    wk8D = nc.dram_tensor("wk8", [128, 16, 512], FP8, kind="ExternalInput").ap()
    xbfD = nc.dram_tensor("xbf", [4, 16, 128, 1024], BF16, kind="ExternalInput").ap()
    wvTD = nc.dram_tensor("wvT", [128, 16, 512], BF16, kind="ExternalInput").ap()
    woTD = nc.dram_tensor("woT", [2, 8, 128, 2048], BF16, kind="ExternalInput").ap()
    cosD = nc.dram_tensor("cosC", [128, T], BF16, kind="ExternalInput").ap()
    sinD = nc.dram_tensor("sinS", [128, T], BF16, kind="ExternalInput").ap()
    out1 = nc.dram_tensor("out1", [T, D], BF16, kind="ExternalOutput").ap()
    out2 = nc.dram_tensor("out2", [T, D], BF16, kind="ExternalOutput").ap()

    EXP = mybir.ActivationFunctionType.Exp

    with tile.TileContext(nc) as tc:
        wq8 = nc.alloc_sbuf_tensor("wq8_sb", [128, 16, 2, 1024], FP8).ap()
        qT = nc.alloc_sbuf_tensor("qT_sb", [128, NQ * T], BF16).ap()
        kT = nc.alloc_sbuf_tensor("kT_sb", [128, NKV * T], BF16).ap()
        vS = nc.alloc_sbuf_tensor("vS_sb", [128, 16 * 256], BF16).ap()
        ctxT = nc.alloc_sbuf_tensor("ctxT_sb", [128, NQ * T], BF16).ap()
        cC = nc.alloc_sbuf_tensor("cosC_sb", [128, T], BF16).ap()
        sS = nc.alloc_sbuf_tensor("sinS_sb", [128, T], BF16).ap()

        for c4 in range(4):
            qsl4 = slice(c4 * 512, (c4 + 1) * 512)
            nc.gpsimd.dma_start(cC[:, qsl4], cosD[:, qsl4])
            nc.gpsimd.dma_start(sS[:, qsl4], sinD[:, qsl4])

        def dma_x8(pool, tb, sp):
            t = pool.tile([128, 2, 2, 512], FP8, tag="x8", name="x8")
            nc.sync.dma_start(t[:], xq8D[tb, sp])
            return t

        # ---------------- Window 1: k (fp8 DR) + v (x-stationary) --------
        x8pool = tc.alloc_tile_pool(name="x8p", bufs=4)
        ropesb = tc.alloc_tile_pool(name="ropesb", bufs=2)
        with tc.tile_pool(name="xba", bufs=6) as xba, \
             tc.tile_pool(name="wvap", bufs=1) as wvap, \
             tc.tile_pool(name="kvp", bufs=1, space="PSUM") as kvp:
            wk8q = [wvap.tile([128, 4, 2, 256], FP8, tag=f"wk8{i}",
                              name=f"wk8{i}") for i in range(4)]
            for c4 in range(4):
                nc.sync.dma_start(wk8q[c4][:], wk8D[:, 4 * c4:4 * (c4 + 1), :])
            x8q = [dma_x8(x8pool, 0, 0), dma_x8(x8pool, 0, 1)]
            wvAq = [wvap.tile([128, 4, 2, 256], BF16, tag=f"wvA{i}",
                              name=f"wvA{i}") for i in range(4)]
            for c4 in range(4):
                nc.sync.dma_start(wvAq[c4][:], wvTD[:, 4 * c4:4 * (c4 + 1), :])
            for tb in range(4):
                if tb == 2:
                    for c8 in range(8):
                        nc.gpsimd.dma_start(wq8[:, 2 * c8:2 * (c8 + 1), :, :],
                                            wq8D[:, 2 * c8:2 * (c8 + 1), :])
                tsl = slice(tb * 512, (tb + 1) * 512)
                kps = [kvp.tile([128, 512], F32, tag=f"kp{j}", bufs=2,
                                name=f"kp{j}") for j in range(2)]
                for sp in range(8):
                    nxt = sp + 2
                    if nxt < 8:
                        x8q.append(dma_x8(x8pool, tb, nxt))
                    elif tb < 3:
                        x8q.append(dma_x8(x8pool, tb + 1, nxt - 8))
                    x8 = x8q.pop(0)
                    for s2 in range(2):
                        s = 2 * sp + s2
                        for j in range(2):
                            nc.tensor.matmul(
                                kps[j][:],
                                wk8q[s // 4][:, s % 4, :, j * 128:(j + 1) * 128],
                                x8[:, s2, :, :], start=(s == 0), stop=(s == 15),
                                perf_mode=DR)
                for j in range(2):
                    _rope_evac(nc, ropesb, kps[j],
                               kT[:, j * T + tb * 512:j * T + (tb + 1) * 512],
                               cC[:, tsl], sS[:, tsl])
                vps = [kvp.tile([128, 512], F32, tag=f"vp{j}", bufs=1,
                                name=f"vp{j}") for j in range(4)]
                xbq = [None, None]
                xbq[0] = xba.tile([128, 2, 512], BF16, tag="xb", name="xb")
                nc.sync.dma_start(xbq[0][:], xbfD[tb, 0])
                xbq[1] = xba.tile([128, 2, 512], BF16, tag="xb", name="xb")
                nc.sync.dma_start(xbq[1][:], xbfD[tb, 1])
                for dp in range(16):
                    if dp + 2 < 16:
                        t_ = xba.tile([128, 2, 512], BF16, tag="xb", name="xb")
                        nc.sync.dma_start(t_[:], xbfD[tb, dp + 2])
                        xbq.append(t_)
                    xbt = xbq.pop(0)
                    for dd in range(2):
                        for sub in range(4):
                            nc.tensor.matmul(
                                vps[sub][:, 0:256],
                                xbt[:, dd, sub * 128:(sub + 1) * 128],
                                wvAq[dp // 4][:, dp % 4, dd, :],
                                start=(dp == 0 and dd == 0),
                                stop=(dp == 15 and dd == 1))
                for sub in range(4):
                    t = tb * 4 + sub
                    nc.vector.tensor_copy(
                        vS[:, t * 256:(t + 1) * 256], vps[sub][:, 0:256])

        # ---------------- Window 2: q heads 0-3 (fp8 DR) -----------------
        with tc.tile_pool(name="qp0", bufs=1, space="PSUM") as qp0:
            x8q = [dma_x8(x8pool, 0, 0), dma_x8(x8pool, 0, 1)]
            for tb in range(4):
                tsl = slice(tb * 512, (tb + 1) * 512)
                qps = [qp0.tile([128, 512], F32, tag=f"qp{m}", bufs=2,
                                name=f"qp{m}") for m in range(4)]
                for sp in range(8):
                    nxt = sp + 2
                    if nxt < 8:
                        x8q.append(dma_x8(x8pool, tb, nxt))
                    elif tb < 3:
                        x8q.append(dma_x8(x8pool, tb + 1, nxt - 8))
                    x8 = x8q.pop(0)
                    for s2 in range(2):
                        s = 2 * sp + s2
                        for m in range(4):
                            nc.tensor.matmul(
                                qps[m][:], wq8[:, s, :, m * 128:(m + 1) * 128],
                                x8[:, s2, :, :], start=(s == 0), stop=(s == 15),
                                perf_mode=DR)
                for m in range(4):
                    _rope_evac(nc, ropesb, qps[m],
                               qT[:, m * T + tb * 512:m * T + (tb + 1) * 512],
                               cC[:, tsl], sS[:, tsl])

        # ---------------- Windows 3+4: attention + q4-7 + o_proj ---------
        def zchain(hp, tb, expP, cxp, attsb):
            """Z-chain for head hp: DVE tree + gpsimd allreduce + recip.
            Returns (cx psum tile, rbs) for the AV/normalize of hp."""
            tA = attsb.tile([128, 2048], BF16, tag="tA", bufs=1, name="tA")
            nc.vector.tensor_add(tA[:], expP[:, 0:2048], expP[:, 2048:4096])
            tB = attsb.tile([128, 2048], BF16, tag="tB", bufs=1, name="tB")
            nc.vector.tensor_add(tB[:], expP[:, 4096:6144], expP[:, 6144:8192])
            tC = attsb.tile([128, 2048], BF16, tag="tC", bufs=1, name="tC")
            nc.vector.tensor_add(tC[:], tA[:], tB[:])
            tD = attsb.tile([128, 1024], BF16, tag="tD", bufs=1, name="tD")
            nc.vector.tensor_add(tD[:], tC[:, 0:1024], tC[:, 1024:2048])
            esum = attsb.tile([128, 512], F32, tag="esum", bufs=2, name="esum")
            nc.vector.tensor_add(esum[:], tD[:, 0:512], tD[:, 512:1024])
            zb = attsb.tile([128, 512], F32, tag="zb", bufs=2, name="zb")
            nc.gpsimd.partition_all_reduce(zb[:], esum[:], 128, ReduceOp.add)
            rbs = attsb.tile([128, 512], F32, tag="rbs", bufs=2, name="rbs")
            nc.vector.reciprocal_approx_fast(rbs[:], zb[:])
            cx = cxp.tile([128, 512], F32, tag="cx", bufs=1, name="cx")
            return cx, rbs

        def av_pair(hp, t2, cx, expP):
            kvp_ = hp // 4
            for u in range(2):
                t = 2 * t2 + u
                nc.tensor.matmul(
                    cx[:],
                    vS[:, t * 256 + kvp_ * 128:t * 256 + (kvp_ + 1) * 128],
                    expP[:, t * 512:(t + 1) * 512],
                    start=(t == 0), stop=(t == 15), skip_group_check=True)

        def attn_zip(h, tb, prev, fillers, scp, cxp, expp, attsb):
            """scores+exp(h) zipped per-sc-tile with AV(prev head) and one
            filler thunk per slot, so no PE mm waits on the exp ring."""
            kv = h // 4
            qsl = qT[:, h * T + tb * 512:h * T + (tb + 1) * 512]
            expT = expp.tile([128, 16 * 512], BF16, tag="expT", name="expT")
            if prev is not None:
                hp, expP = prev
                cx, rbs = zchain(hp, tb, expP, cxp, attsb)
            for t2 in range(8):
                sc = scp.tile([128, 1024], F32, tag="sc", bufs=2, name="sc")
                for u in range(2):
                    t = 2 * t2 + u
                    nc.tensor.matmul(
                        sc[:, u * 512:(u + 1) * 512],
                        kT[:, kv * T + t * 128:kv * T + (t + 1) * 128],
                        qsl, start=True, stop=True, skip_group_check=True)
                if prev is not None:
                    av_pair(prev[0], t2, cx, prev[1])
                if fillers:
                    fillers.popleft()()
                nc.scalar.activation(
                    expT[:, t2 * 1024:(t2 + 1) * 1024], sc[:], EXP,
                    scale=ESCALE)
            if prev is not None:
                hp = prev[0]
                nc.vector.tensor_mul(
                    ctxT[:, hp * T + tb * 512:hp * T + (tb + 1) * 512],
                    cx[:], rbs[:])
            return expT

        def attn_tail(h, tb, expT, fillers, cxp, attsb):
            """AV + normalize for the window's last head, with fillers."""
            cx, rbs = zchain(h, tb, expT, cxp, attsb)
            for t2 in range(8):
                av_pair(h, t2, cx, expT)
                if fillers:
                    fillers.popleft()()
            nc.vector.tensor_mul(
                ctxT[:, h * T + tb * 512:h * T + (tb + 1) * 512],
                cx[:], rbs[:])

        def dma_wot(hh0, eb, wotp):
            wot = wotp.tile([128, 4, 512], BF16, tag="wot", name="wot")
            nc.sync.dma_start(wot[:], woTD[hh0 // 4, eb])
            return wot

        def oproj_stream(groups, pop, wotp, osbp, tag="po"):
            """Thunk stream for o_proj groups [(hh0, tb, eb, outD, evac,
            dma)]: wot prefetched one group ahead; 4 mm-thunks per group."""
            thunks = []
            cells = [dict() for _ in groups]

            def mk_pf(idx):
                def pf():
                    hh0, tb, eb = groups[idx][:3]
                    cells[idx]["wot"] = dma_wot(hh0, eb, wotp)
                return pf

            def mk_mm(idx, sub):
                def mm():
                    hh0, tb, eb, outD, evac, dma = groups[idx]
                    wot = cells[idx]["wot"]
                    po = pop.tile([128, 512], F32, tag=tag, bufs=3, name=tag)
                    for i in range(4):
                        c0 = (hh0 + i) * T + tb * 512 + sub * 128
                        nc.tensor.matmul(
                            po[:], ctxT[:, c0:c0 + 128], wot[:, i, :],
                            start=(i == 0), stop=(i == 3),
                            skip_group_check=True)
                    ot = osbp.tile([128, 512], BF16, tag="ot", name="ot")
                    if sub % 2 == 0:
                        nc.scalar.copy(ot[:], po[:])
                        nc.scalar.dma_start(
                            outD[tb * 512 + sub * 128:
                                 tb * 512 + (sub + 1) * 128,
                                 eb * 512:(eb + 1) * 512], ot[:])
                    else:
                        evac(ot[:], po[:])
                        dma(outD[tb * 512 + sub * 128:
                                 tb * 512 + (sub + 1) * 128,
                                 eb * 512:(eb + 1) * 512], ot[:])
                return mm

            for idx in range(len(groups)):
                if idx == 0:
                    thunks.append(mk_pf(0))
                for sub in range(4):
                    if sub == 2 and idx + 1 < len(groups):
                        thunks.append(mk_pf(idx + 1))
                    thunks.append(mk_mm(idx, sub))
            return thunks

        from collections import deque

        with tc.tile_pool(name="expp", bufs=2) as expp, \
             tc.tile_pool(name="attsb", bufs=1) as attsb, \
             tc.tile_pool(name="scp", bufs=1, space="PSUM") as scp, \
             tc.tile_pool(name="cxp", bufs=1, space="PSUM") as cxp:
            # ---- Window 3: attn h0-3 zipped with q-proj h4-7 + o_proj ---
            with tc.tile_pool(name="wotp3", bufs=2) as wotp3, \
                 tc.tile_pool(name="osbp3", bufs=2) as osbp3, \
                 tc.tile_pool(name="qp1", bufs=1, space="PSUM") as qp1:
                for tb in range(4):
                    tsl = slice(tb * 512, (tb + 1) * 512)

                    def qchunk_thunks(m, tb=tb, tsl=tsl):
                        """8 thunks: 2 DR mms each (one s-pair); rope on
                        the last."""
                        qcell = {}
                        ths = []

                        def mk(sp, m=m, tb=tb, tsl=tsl):
                            def th():
                                if sp == 0:
                                    qcell["qp"] = qp1.tile(
                                        [128, 512], F32, tag="qp", bufs=3,
                                        name="qp")
                                    qcell["q"] = [dma_x8(x8pool, tb, 0),
                                                  dma_x8(x8pool, tb, 1)]
                                qp = qcell["qp"]
                                if sp + 2 < 8:
                                    qcell["q"].append(
                                        dma_x8(x8pool, tb, sp + 2))
                                x8 = qcell["q"].pop(0)
                                for s2 in range(2):
                                    s = 2 * sp + s2
                                    nc.tensor.matmul(
                                        qp[:],
                                        wq8[:, s, :, m * 128:(m + 1) * 128],
                                        x8[:, s2, :, :],
                                        start=(s == 0), stop=(s == 15),
                                        perf_mode=DR, skip_group_check=True)
                                if sp == 7:
                                    _rope_evac(
                                        nc, ropesb, qp,
                                        qT[:, m * T + tb * 512:
                                           m * T + (tb + 1) * 512],
                                        cC[:, tsl], sS[:, tsl])
                            return th
                        for sp in range(8):
                            ths.append(mk(sp))
                        return ths

                    fillers = deque()
                    for m in (4, 5, 6, 7):
                        fillers.extend(qchunk_thunks(m))
                    if tb > 0:
                        groups = [(0, tb - 1, eb, out1,
                                   nc.vector.tensor_copy,
                                   nc.gpsimd.dma_start) for eb in (0, 1)]
                        fillers.extend(oproj_stream(
                            groups, qp1, wotp3, osbp3, tag="qp"))
                    e0 = attn_zip(0, tb, None, fillers, scp, cxp, expp, attsb)
                    e1 = attn_zip(1, tb, (0, e0), fillers, scp, cxp, expp, attsb)
                    e2 = attn_zip(2, tb, (1, e1), fillers, scp, cxp, expp, attsb)
                    e3 = attn_zip(3, tb, (2, e2), fillers, scp, cxp, expp, attsb)
                    attn_tail(3, tb, e3, fillers, cxp, attsb)
                    while fillers:
                        fillers.popleft()()

            # ---- Window 4: attn h4-7 zipped with o_proj drains ----------
            with tc.tile_pool(name="wotp", bufs=2) as wotp, \
                 tc.tile_pool(name="osbp", bufs=4) as osbp, \
                 tc.tile_pool(name="pop", bufs=1, space="PSUM") as pop:
                for tb in range(4):
                    ebs1 = list(range(2, 8)) if tb < 3 else list(range(8))
                    groups = [(0, tb, eb, out1, nc.vector.tensor_copy,
                               nc.gpsimd.dma_start) for eb in ebs1]
                    if tb > 0:
                        groups += [(4, tb - 1, eb, out2,
                                    nc.vector.tensor_copy,
                                    nc.gpsimd.dma_start) for eb in range(8)]
                    fillers = deque(oproj_stream(groups, pop, wotp, osbp))
                    e4 = attn_zip(4, tb, None, fillers, scp, cxp, expp, attsb)
                    e5 = attn_zip(5, tb, (4, e4), fillers, scp, cxp, expp, attsb)
                    e6 = attn_zip(6, tb, (5, e5), fillers, scp, cxp, expp, attsb)
                    e7 = attn_zip(7, tb, (6, e6), fillers, scp, cxp, expp, attsb)
                    attn_tail(7, tb, e7, fillers, cxp, attsb)
                    while fillers:
                        fillers.popleft()()

                # ---- Window 5: o_proj(h4-7, tb=3) -----------------------
                groups = [(4, 3, eb, out2, nc.scalar.copy,
                           nc.scalar.dma_start) for eb in range(8)]
                for th in oproj_stream(groups, pop, wotp, osbp):
                    th()

        ropesb.release()
        x8pool.release()
    nc.compile()
    _CACHE["nc"] = nc
    return nc


def _prep_inputs(x, wq, wk, wv, wo, freqs_cos, freqs_sin):
    bf = ml_dtypes.bfloat16
    f8 = ml_dtypes.float8_e4m3fn
    perm = np.concatenate([np.arange(0, 128, 2), np.arange(1, 128, 2)])

    def permute_heads(w):
        nh = w.shape[0] // 128
        return w.reshape(nh, 128, D)[:, perm, :].reshape(nh * 128, D)

    def pack_w8(w):
        # w [M, 4096] -> [128, 16, 2*M]: [p, s, j*M+m] = w[m, 256s+128j+p]
        M = w.shape[0]
        wt = np.ascontiguousarray(w.T).reshape(16, 2, 128, M)
        return np.ascontiguousarray(
            wt.transpose(2, 0, 1, 3).reshape(128, 16, 2 * M).astype(f8))

    cosC = np.ascontiguousarray(np.tile(freqs_cos.T, (2, 1)), dtype=bf)
    sinS = np.ascontiguousarray(
        np.concatenate([-freqs_sin.T, freqs_sin.T], axis=0), dtype=bf)

    in_maps = []
    for c in range(8):
        b, g = c // NG, c % NG
        wq_g = permute_heads(wq[g * NQ * HD:(g + 1) * NQ * HD]) * WS
        wk_g = permute_heads(wk[g * NKV * HD:(g + 1) * NKV * HD]) * WS
        wv_g = wv[g * NKV * HD:(g + 1) * NKV * HD]
        # x8 [4,8,128,2048]: [tb,sp,p,s2*1024+j*512+n]
        #   = 16*x[b, 512tb+n, 256*(2sp+s2)+128j+p]
        xs = (x[b] * XS).T.reshape(8, 2, 2, 128, 4, 512)
        xq8 = np.ascontiguousarray(
            xs.transpose(4, 0, 3, 1, 2, 5).reshape(4, 8, 128, 2048).astype(f8))
        # xbf [4,16,128,1024]: [tb,dp,p,dd*512+n] = x[b, 512tb+n, 256dp+128dd+p]
        xbf = np.ascontiguousarray(
            x[b].T.reshape(16, 2, 128, 4, 512).transpose(3, 0, 2, 1, 4)
            .reshape(4, 16, 128, 1024).astype(bf))
        # wvT [128,16,512]: [p,dp,dd*256+m] = wv_g[m, 256dp+128dd+p]
        wvp = np.ascontiguousarray(
            wv_g.T.reshape(16, 2, 128, 256).transpose(2, 0, 1, 3)
            .reshape(128, 16, 512).astype(bf))
        # woT [2,8,128,2048]: [half,eb,p,i*512+c] = wo[eb*512+c, g off + (4half+i)*128+p]
        woT = wo[:, g * NQ * HD:(g + 1) * NQ * HD].T  # [1024, 4096]
        wop = np.ascontiguousarray(
            woT.reshape(2, 4, 128, 8, 512).transpose(0, 3, 2, 1, 4)
            .reshape(2, 8, 128, 2048).astype(bf))
        in_maps.append({
            "xq8": xq8,
            "wq8": pack_w8(wq_g),
            "wk8": pack_w8(wk_g),
            "xbf": xbf,
            "wvT": wvp,
            "woT": wop,
            "cosC": cosC, "sinS": sinS,
        })
    return in_maps


def kernel(x, wq, wk, wv, wo, freqs_cos, freqs_sin, start_pos=0, _trace=False):
    x = np.asarray(x, dtype=np.float32)
    wq = np.asarray(wq, np.float32)
    wk = np.asarray(wk, np.float32)
    wv = np.asarray(wv, np.float32)
    wo = np.asarray(wo, np.float32)
    freqs_cos = np.asarray(freqs_cos, np.float32)
    freqs_sin = np.asarray(freqs_sin, np.float32)

    nc = _build()
    in_maps = _prep_inputs(x, wq, wk, wv, wo, freqs_cos, freqs_sin)
    try:
        res = run_bass_kernel_spmd(nc, in_maps, core_ids=list(range(8)),
                                   trace=_trace)
    except ModuleNotFoundError:
        res = run_bass_kernel_spmd(nc, in_maps, core_ids=list(range(8)),
                                   trace=False)
    out = np.zeros((BT, T, D), np.float32)
    for c in range(8):
        out[c // NG] += np.asarray(res.results[c]["out1"], np.float32)
        out[c // NG] += np.asarray(res.results[c]["out2"], np.float32)
    if _trace:
        kernel.last_results = res
    return out


# revision 24
# speedup vs baseline: 1.1888x; 1.1888x over previous
"""Trainium2 Bass kernel for GQA attention prefill (B=2,T=2048,D=4096,H=32,KVH=8).

Sharding: data-parallel over batch (2) x tensor-parallel over heads (4 groups
of 8 q-heads / 2 kv-heads). 8 cores. Each core emits TWO partial o_proj
outputs (head-halves 0-3 / 4-7); host sums the 8 partials per batch.

v2 design (vs v1 baseline 1.03ms):
  - q/k projections in fp8 (e4m3) with DoubleRow matmuls (256-deep
    contraction per pass). x scaled by 16, wq/wk by 64; the 2^20 factor on
    scores is folded into the exp() activation scale.
  - v projection x-stationary -> vS produced directly in [tok, hd] layout
    (no PE transpose pass).
  - softmax denominator via DVE add-chain over exp tiles + gpsimd
    partition-reduce + fast reciprocal (replaces 109us of ones-matmuls).
  - o_proj split by head-halves; its matmuls are issued interleaved with
    the second attention window so the PE array stays busy while Scalar
    does exp(); remaining windows interleave q-proj with attention.

Per-core layouts:
  xq8D  [4,16,128,1024] fp8  x8[tb,s,p,j*512+n]  = 16*x[b, 512tb+n, 256s+128j+p]
  wq8D  [16,128,2048]   fp8  [s,p,j*1024+m]      = 64*wq_perm[m, 256s+128j+p]
  wk8D  [16,128,512]    fp8  [s,p,j*256+m]       = 64*wk_perm[m, ...]
  xbfD  [4096,2048]     bf16 x[b].T
  wvTD  [4096,256]      bf16
  woTD  [1024,4096]     bf16
  cosC/sinS [128,2048]  bf16 rope tables (rows 0:64 even dims, 64:128 odd;
                             sinS rows 0:64 negated)
"""

import numpy as np
import ml_dtypes

import concourse.bass as bass
import concourse.tile as tile
from concourse import bacc, mybir
from concourse.bass_isa import ReduceOp
from concourse.bass_utils import run_bass_kernel_spmd

BF16 = mybir.dt.bfloat16
F32 = mybir.dt.float32
FP8 = mybir.dt.float8e4
BT, T, D = 2, 2048, 4096
H, KVH, HD = 32, 8, 128
NQ, NKV = 8, 2          # per-core q heads / kv heads
NG = 4                  # head groups
SCALE = 1.0 / np.sqrt(128.0)
XS, WS = 16.0, 64.0     # fp8 scale factors for x and wq/wk
ESCALE = float(SCALE / (XS * XS * WS * WS))
DR = mybir.MatmulPerfMode.DoubleRow

_CACHE = {}


def _rope_evac(nc, sb, ps, out_sl, c_sl, s_sl):
    """ps: PSUM [128,512] f32 -> out_sl: SBUF bf16 [128,512] with RoPE.
    Rows 0:64 = even dims, 64:128 = odd dims (host-permuted weights).
    out = ps*C + shift64(ps)*S, via partition-shifted DVE reads."""
    tmp = sb.tile([128, 512], F32, tag="rtmp", name="rtmp")
    nc.vector.tensor_mul(tmp[0:64, :], ps[64:128, :], s_sl[0:64, :])
    nc.vector.tensor_mul(tmp[64:128, :], ps[0:64, :], s_sl[64:128, :])
    tmp2 = sb.tile([128, 512], F32, tag="rtmp2", name="rtmp2")
    nc.vector.tensor_mul(tmp2[:], ps[:], c_sl)
    nc.vector.tensor_add(out_sl, tmp2[:], tmp[:])


def _build():
    if "nc" in _CACHE:
        return _CACHE["nc"]
    nc = bacc.Bacc("TRN2", target_bir_lowering=False, debug=False, num_devices=8)
    xq8D = nc.dram_tensor("xq8", [4, 8, 128, 2048], FP8, kind="ExternalInput").ap()
    wq8D = nc.dram_tensor("wq8", [128, 16, 2048], FP8, kind="ExternalInput").ap()
    wk8D = nc.dram_tensor("wk8", [128, 16, 512], FP8, kind="ExternalInput").ap()
    xbfD = nc.dram_tensor("xbf", [4, 16, 128, 1024], BF16, kind="ExternalInput").ap()
    wvTD = nc.dram_tensor("wvT", [128, 16, 512], BF16, kind="ExternalInput").ap()
    woTD = nc.dram_tensor("woT", [2, 8, 128, 2048], BF16, kind="ExternalInput").ap()
    cosD = nc.dram_tensor("cosC", [128, T], BF16, kind="ExternalInput").ap()
    sinD = nc.dram_tensor("sinS", [128, T], BF16, kind="ExternalInput").ap()
    out1 = nc.dram_tensor("out1", [T, D], BF16, kind="ExternalOutput").ap()
    out2 = nc.dram_tensor("out2", [T, D], BF16, kind="ExternalOutput").ap()

    EXP = mybir.ActivationFunctionType.Exp

    with tile.TileContext(nc) as tc:
        wq8 = nc.alloc_sbuf_tensor("wq8_sb", [128, 16, 2, 1024], FP8).ap()
        qT = nc.alloc_sbuf_tensor("qT_sb", [128, NQ * T], BF16).ap()
        kT = nc.alloc_sbuf_tensor("kT_sb", [128, NKV * T], BF16).ap()
        vS = nc.alloc_sbuf_tensor("vS_sb", [128, 16 * 256], BF16).ap()
        ctxT = nc.alloc_sbuf_tensor("ctxT_sb", [128, NQ * T], BF16).ap()
        cC = nc.alloc_sbuf_tensor("cosC_sb", [128, T], BF16).ap()
        sS = nc.alloc_sbuf_tensor("sinS_sb", [128, T], BF16).ap()

        for c4 in range(4):
            qsl4 = slice(c4 * 512, (c4 + 1) * 512)
            nc.gpsimd.dma_start(cC[:, qsl4], cosD[:, qsl4])
            nc.gpsimd.dma_start(sS[:, qsl4], sinD[:, qsl4])

        def dma_x8(pool, tb, sp):
            t = pool.tile([128, 2, 2, 512], FP8, tag="x8", name="x8")
            nc.sync.dma_start(t[:], xq8D[tb, sp])
            return t

        # ---------------- Window 1: k (fp8 DR) + v (x-stationary) --------
        x8pool = tc.alloc_tile_pool(name="x8p", bufs=4)
        ropesb = tc.alloc_tile_pool(name="ropesb", bufs=2)
        with tc.tile_pool(name="xba", bufs=6) as xba, \
             tc.tile_pool(name="wvap", bufs=1) as wvap, \
             tc.tile_pool(name="kvp", bufs=1, space="PSUM") as kvp:
            wk8q = [wvap.tile([128, 4, 2, 256], FP8, tag=f"wk8{i}",
                              name=f"wk8{i}") for i in range(4)]
            for c4 in range(4):
                nc.sync.dma_start(wk8q[c4][:], wk8D[:, 4 * c4:4 * (c4 + 1), :])
            x8q = [dma_x8(x8pool, 0, 0), dma_x8(x8pool, 0, 1)]
            wvAq = [wvap.tile([128, 4, 2, 256], BF16, tag=f"wvA{i}",
                              name=f"wvA{i}") for i in range(4)]
            for c4 in range(4):
                nc.sync.dma_start(wvAq[c4][:], wvTD[:, 4 * c4:4 * (c4 + 1), :])
            for tb in range(4):
                if tb == 2:
                    for c8 in range(8):
                        nc.gpsimd.dma_start(wq8[:, 2 * c8:2 * (c8 + 1), :, :],
                                            wq8D[:, 2 * c8:2 * (c8 + 1), :])
                tsl = slice(tb * 512, (tb + 1) * 512)
                kps = [kvp.tile([128, 512], F32, tag=f"kp{j}", bufs=2,
                                name=f"kp{j}") for j in range(2)]
                for sp in range(8):
                    nxt = sp + 2
                    if nxt < 8:
                        x8q.append(dma_x8(x8pool, tb, nxt))
                    elif tb < 3:
                        x8q.append(dma_x8(x8pool, tb + 1, nxt - 8))
                    x8 = x8q.pop(0)
                    for s2 in range(2):
                        s = 2 * sp + s2
                        for j in range(2):
                            nc.tensor.matmul(
                                kps[j][:],
                                wk8q[s // 4][:, s % 4, :, j * 128:(j + 1) * 128],
                                x8[:, s2, :, :], start=(s == 0), stop=(s == 15),
                                perf_mode=DR)
                for j in range(2):
                    _rope_evac(nc, ropesb, kps[j],
                               kT[:, j * T + tb * 512:j * T + (tb + 1) * 512],
                               cC[:, tsl], sS[:, tsl])
                vps = [kvp.tile([128, 512], F32, tag=f"vp{j}", bufs=1,
                                name=f"vp{j}") for j in range(4)]
                xbq = [None, None]
                xbq[0] = xba.tile([128, 2, 512], BF16, tag="xb", name="xb")
                nc.sync.dma_start(xbq[0][:], xbfD[tb, 0])
                xbq[1] = xba.tile([128, 2, 512], BF16, tag="xb", name="xb")
                nc.sync.dma_start(xbq[1][:], xbfD[tb, 1])
                for dp in range(16):
                    if dp + 2 < 16:
                        t_ = xba.tile([128, 2, 512], BF16, tag="xb", name="xb")
                        nc.sync.dma_start(t_[:], xbfD[tb, dp + 2])
                        xbq.append(t_)
                    xbt = xbq.pop(0)
                    for dd in range(2):
                        for sub in range(4):
                            nc.tensor.matmul(
                                vps[sub][:, 0:256],
                                xbt[:, dd, sub * 128:(sub + 1) * 128],
                                wvAq[dp // 4][:, dp % 4, dd, :],
                                start=(dp == 0 and dd == 0),
                                stop=(dp == 15 and dd == 1))
                for sub in range(4):
                    t = tb * 4 + sub
                    nc.vector.tensor_copy(
                        vS[:, t * 256:(t + 1) * 256], vps[sub][:, 0:256])

        # ---------------- Window 2: q heads 0-3 (fp8 DR) -----------------
        with tc.tile_pool(name="qp0", bufs=1, space="PSUM") as qp0:
            x8q = [dma_x8(x8pool, 0, 0), dma_x8(x8pool, 0, 1)]
            for tb in range(4):
                tsl = slice(tb * 512, (tb + 1) * 512)
                qps = [qp0.tile([128, 512], F32, tag=f"qp{m}", bufs=2,
                                name=f"qp{m}") for m in range(4)]
                for sp in range(8):
                    nxt = sp + 2
                    if nxt < 8:
                        x8q.append(dma_x8(x8pool, tb, nxt))
                    elif tb < 3:
                        x8q.append(dma_x8(x8pool, tb + 1, nxt - 8))
                    x8 = x8q.pop(0)
                    for s2 in range(2):
                        s = 2 * sp + s2
                        for m in range(4):
                            nc.tensor.matmul(
                                qps[m][:], wq8[:, s, :, m * 128:(m + 1) * 128],
                                x8[:, s2, :, :], start=(s == 0), stop=(s == 15),
                                perf_mode=DR)
                for m in range(4):
                    _rope_evac(nc, ropesb, qps[m],
                               qT[:, m * T + tb * 512:m * T + (tb + 1) * 512],
                               cC[:, tsl], sS[:, tsl])

        # ---------------- Windows 3+4: attention + q4-7 + o_proj ---------
        def zchain(hp, tb, expP, cxp, attsb):
            """Z-chain for head hp: DVE tree + gpsimd allreduce + recip.
            Returns (cx psum tile, rbs) for the AV/normalize of hp."""
            tA = attsb.tile([128, 2048], BF16, tag="tA", bufs=1, name="tA")
            nc.vector.tensor_add(tA[:], expP[:, 0:2048], expP[:, 2048:4096])
            tB = attsb.tile([128, 2048], BF16, tag="tB", bufs=1, name="tB")
            nc.vector.tensor_add(tB[:], expP[:, 4096:6144], expP[:, 6144:8192])
            tC = attsb.tile([128, 2048], BF16, tag="tC", bufs=1, name="tC")
            nc.vector.tensor_add(tC[:], tA[:], tB[:])
            tD = attsb.tile([128, 1024], BF16, tag="tD", bufs=1, name="tD")
            nc.vector.tensor_add(tD[:], tC[:, 0:1024], tC[:, 1024:2048])
            esum = attsb.tile([128, 512], F32, tag="esum", bufs=2, name="esum")
            nc.vector.tensor_add(esum[:], tD[:, 0:512], tD[:, 512:1024])
            zb = attsb.tile([128, 512], F32, tag="zb", bufs=2, name="zb")
            nc.gpsimd.partition_all_reduce(zb[:], esum[:], 128, ReduceOp.add)
            rbs = attsb.tile([128, 512], F32, tag="rbs", bufs=2, name="rbs")
            nc.vector.reciprocal_approx_fast(rbs[:], zb[:])
            cx = cxp.tile([128, 512], F32, tag="cx", bufs=2, name="cx")
            return cx, rbs

        def av_pair(hp, t2, cx, expP):
            kvp_ = hp // 4
            for u in range(2):
                t = 2 * t2 + u
                nc.tensor.matmul(
                    cx[:],
                    vS[:, t * 256 + kvp_ * 128:t * 256 + (kvp_ + 1) * 128],
                    expP[:, t * 512:(t + 1) * 512],
                    start=(t == 0), stop=(t == 15), skip_group_check=True)

        def attn_zip(h, tb, prev, fillers, scp, cxp, expp, attsb):
            """scores+exp(h) zipped per-sc-tile with AV(prev head) and one
            filler thunk per slot, so no PE mm waits on the exp ring."""
            kv = h // 4
            qsl = qT[:, h * T + tb * 512:h * T + (tb + 1) * 512]
            expT = expp.tile([128, 16 * 512], BF16, tag="expT", name="expT")
            if prev is not None:
                hp, expP = prev
                cx, rbs = zchain(hp, tb, expP, cxp, attsb)
            for t2 in range(8):
                sc = scp.tile([128, 1024], F32, tag="sc", bufs=2, name="sc")
                for u in range(2):
                    t = 2 * t2 + u
                    nc.tensor.matmul(
                        sc[:, u * 512:(u + 1) * 512],
                        kT[:, kv * T + t * 128:kv * T + (t + 1) * 128],
                        qsl, start=True, stop=True, skip_group_check=True)
                if prev is not None:
                    av_pair(prev[0], t2, cx, prev[1])
                if fillers:
                    fillers.popleft()()
                nc.scalar.activation(
                    expT[:, t2 * 1024:(t2 + 1) * 1024], sc[:], EXP,
                    scale=ESCALE)
            if prev is not None:
                hp = prev[0]
                nc.vector.tensor_mul(
                    ctxT[:, hp * T + tb * 512:hp * T + (tb + 1) * 512],
                    cx[:], rbs[:])
            return expT

        def attn_tail(h, tb, expT, fillers, cxp, attsb):
            """AV + normalize for the window's last head, with fillers."""
            cx, rbs = zchain(h, tb, expT, cxp, attsb)
            for t2 in range(8):
                av_pair(h, t2, cx, expT)
                if fillers:
                    fillers.popleft()()
            nc.vector.tensor_mul(
                ctxT[:, h * T + tb * 512:h * T + (tb + 1) * 512],
                cx[:], rbs[:])

        def dma_wot(hh0, eb, wotp):
            wot = wotp.tile([128, 4, 512], BF16, tag="wot", name="wot")
            nc.sync.dma_start(wot[:], woTD[hh0 // 4, eb])
            return wot

        def oproj_stream(groups, pop, wotp, osbp, tag="po"):
            """Thunk stream for o_proj groups [(hh0, tb, eb, outD, evac,
            dma)]: wot prefetched one group ahead; 4 mm-thunks per group."""
            thunks = []
            cells = [dict() for _ in groups]

            def mk_pf(idx):
                def pf():
                    hh0, tb, eb = groups[idx][:3]
                    cells[idx]["wot"] = dma_wot(hh0, eb, wotp)
                return pf

            def mk_mm(idx, sub):
                def mm():
                    hh0, tb, eb, outD, evac, dma = groups[idx]
                    wot = cells[idx]["wot"]
                    po = pop.tile([128, 512], F32, tag=tag, bufs=2, name=tag)
                    for i in range(4):
                        c0 = (hh0 + i) * T + tb * 512 + sub * 128
                        nc.tensor.matmul(
                            po[:], ctxT[:, c0:c0 + 128], wot[:, i, :],
                            start=(i == 0), stop=(i == 3),
                            skip_group_check=True)
                    ot = osbp.tile([128, 512], BF16, tag="ot", name="ot")
                    if sub % 2 == 0:
                        nc.scalar.copy(ot[:], po[:])
                        nc.scalar.dma_start(
                            outD[tb * 512 + sub * 128:
                                 tb * 512 + (sub + 1) * 128,
                                 eb * 512:(eb + 1) * 512], ot[:])
                    else:
                        evac(ot[:], po[:])
                        dma(outD[tb * 512 + sub * 128:
                                 tb * 512 + (sub + 1) * 128,
                                 eb * 512:(eb + 1) * 512], ot[:])
                return mm

            for idx in range(len(groups)):
                if idx == 0:
                    thunks.append(mk_pf(0))
                for sub in range(4):
                    if sub == 2 and idx + 1 < len(groups):
                        thunks.append(mk_pf(idx + 1))
                    thunks.append(mk_mm(idx, sub))
            return thunks

        from collections import deque

        with tc.tile_pool(name="expp", bufs=2) as expp, \
             tc.tile_pool(name="attsb", bufs=1) as attsb, \
             tc.tile_pool(name="scp", bufs=1, space="PSUM") as scp, \
             tc.tile_pool(name="cxp", bufs=1, space="PSUM") as cxp:
            # ---- Window 3: attn h0-3 zipped with q-proj h4-7 + o_proj ---
            with tc.tile_pool(name="wotp3", bufs=2) as wotp3, \
                 tc.tile_pool(name="osbp3", bufs=2) as osbp3, \
                 tc.tile_pool(name="qp1", bufs=1, space="PSUM") as qp1:
                for tb in range(4):
                    tsl = slice(tb * 512, (tb + 1) * 512)

                    def qchunk_thunks(m, tb=tb, tsl=tsl):
                        """8 thunks: 2 DR mms each (one s-pair); rope on
                        the last."""
                        qcell = {}
                        ths = []

                        def mk(sp, m=m, tb=tb, tsl=tsl):
                            def th():
                                if sp == 0:
                                    qcell["qp"] = qp1.tile(
                                        [128, 512], F32, tag="qp", bufs=2,
                                        name="qp")
                                    qcell["q"] = [dma_x8(x8pool, tb, 0),
                                                  dma_x8(x8pool, tb, 1)]
                                qp = qcell["qp"]
                                if sp + 2 < 8:
                                    qcell["q"].append(
                                        dma_x8(x8pool, tb, sp + 2))
                                x8 = qcell["q"].pop(0)
                                for s2 in range(2):
                                    s = 2 * sp + s2
                                    nc.tensor.matmul(
                                        qp[:],
                                        wq8[:, s, :, m * 128:(m + 1) * 128],
                                        x8[:, s2, :, :],
                                        start=(s == 0), stop=(s == 15),
                                        perf_mode=DR, skip_group_check=True)
                                if sp == 7:
                                    _rope_evac(
                                        nc, ropesb, qp,
                                        qT[:, m * T + tb * 512:
                                           m * T + (tb + 1) * 512],
                                        cC[:, tsl], sS[:, tsl])
                            return th
                        for sp in range(8):
                            ths.append(mk(sp))
                        return ths

                    fillers = deque()
                    for m in (4, 5, 6, 7):
                        fillers.extend(qchunk_thunks(m))
                    if tb > 0:
                        groups = [(0, tb - 1, eb, out1,
                                   nc.vector.tensor_copy,
                                   nc.gpsimd.dma_start) for eb in (0, 1)]
                        fillers.extend(oproj_stream(
                            groups, qp1, wotp3, osbp3, tag="qp"))
                    e0 = attn_zip(0, tb, None, fillers, scp, cxp, expp, attsb)
                    e1 = attn_zip(1, tb, (0, e0), fillers, scp, cxp, expp, attsb)
                    e2 = attn_zip(2, tb, (1, e1), fillers, scp, cxp, expp, attsb)
                    e3 = attn_zip(3, tb, (2, e2), fillers, scp, cxp, expp, attsb)
                    attn_tail(3, tb, e3, fillers, cxp, attsb)
                    while fillers:
                        fillers.popleft()()

            # ---- Window 4: attn h4-7 zipped with o_proj drains ----------
            with tc.tile_pool(name="wotp", bufs=2) as wotp, \
                 tc.tile_pool(name="osbp", bufs=4) as osbp, \
                 tc.tile_pool(name="pop", bufs=1, space="PSUM") as pop:
                for tb in range(4):
                    ebs1 = list(range(2, 8)) if tb < 3 else list(range(8))
                    groups = [(0, tb, eb, out1, nc.vector.tensor_copy,
                               nc.gpsimd.dma_start) for eb in ebs1]
                    if tb > 0:
                        groups += [(4, tb - 1, eb, out2,
                                    nc.vector.tensor_copy,
                                    nc.gpsimd.dma_start) for eb in range(8)]
                    fillers = deque(oproj_stream(groups, pop, wotp, osbp))
                    e4 = attn_zip(4, tb, None, fillers, scp, cxp, expp, attsb)
                    e5 = attn_zip(5, tb, (4, e4), fillers, scp, cxp, expp, attsb)
                    e6 = attn_zip(6, tb, (5, e5), fillers, scp, cxp, expp, attsb)
                    e7 = attn_zip(7, tb, (6, e6), fillers, scp, cxp, expp, attsb)
                    attn_tail(7, tb, e7, fillers, cxp, attsb)
                    while fillers:
                        fillers.popleft()()

                # ---- Window 5: o_proj(h4-7, tb=3) -----------------------
                groups = [(4, 3, eb, out2, nc.scalar.copy,
                           nc.scalar.dma_start) for eb in range(8)]
                for th in oproj_stream(groups, pop, wotp, osbp):
                    th()

        ropesb.release()
        x8pool.release()
    nc.compile()
    _CACHE["nc"] = nc
    return nc


def _prep_inputs(x, wq, wk, wv, wo, freqs_cos, freqs_sin):
    bf = ml_dtypes.bfloat16
    f8 = ml_dtypes.float8_e4m3fn
    perm = np.concatenate([np.arange(0, 128, 2), np.arange(1, 128, 2)])

    def permute_heads(w):
        nh = w.shape[0] // 128
        return w.reshape(nh, 128, D)[:, perm, :].reshape(nh * 128, D)

    def pack_w8(w):
        # w [M, 4096] -> [128, 16, 2*M]: [p, s, j*M+m] = w[m, 256s+128j+p]
        M = w.shape[0]
        wt = np.ascontiguousarray(w.T).reshape(16, 2, 128, M)
        return np.ascontiguousarray(
            wt.transpose(2, 0, 1, 3).reshape(128, 16, 2 * M).astype(f8))

    cosC = np.ascontiguousarray(np.tile(freqs_cos.T, (2, 1)), dtype=bf)
    sinS = np.ascontiguousarray(
        np.concatenate([-freqs_sin.T, freqs_sin.T], axis=0), dtype=bf)

    in_maps = []
    for c in range(8):
        b, g = c // NG, c % NG
        wq_g = permute_heads(wq[g * NQ * HD:(g + 1) * NQ * HD]) * WS
        wk_g = permute_heads(wk[g * NKV * HD:(g + 1) * NKV * HD]) * WS
        wv_g = wv[g * NKV * HD:(g + 1) * NKV * HD]
        # x8 [4,8,128,2048]: [tb,sp,p,s2*1024+j*512+n]
        #   = 16*x[b, 512tb+n, 256*(2sp+s2)+128j+p]
        xs = (x[b] * XS).T.reshape(8, 2, 2, 128, 4, 512)
        xq8 = np.ascontiguousarray(
            xs.transpose(4, 0, 3, 1, 2, 5).reshape(4, 8, 128, 2048).astype(f8))
        # xbf [4,16,128,1024]: [tb,dp,p,dd*512+n] = x[b, 512tb+n, 256dp+128dd+p]
        xbf = np.ascontiguousarray(
            x[b].T.reshape(16, 2, 128, 4, 512).transpose(3, 0, 2, 1, 4)
            .reshape(4, 16, 128, 1024).astype(bf))
        # wvT [128,16,512]: [p,dp,dd*256+m] = wv_g[m, 256dp+128dd+p]
        wvp = np.ascontiguousarray(
            wv_g.T.reshape(16, 2, 128, 256).transpose(2, 0, 1, 3)
            .reshape(128, 16, 512).astype(bf))
        # woT [2,8,128,2048]: [half,eb,p,i*512+c] = wo[eb*512+c, g off + (4half+i)*128+p]
        woT = wo[:, g * NQ * HD:(g + 1) * NQ * HD].T  # [1024, 4096]
        wop = np.ascontiguousarray(
            woT.reshape(2, 4, 128, 8, 512).transpose(0, 3, 2, 1, 4)
            .reshape(2, 8, 128, 2048).astype(bf))
        in_maps.append({
            "xq8": xq8,
            "wq8": pack_w8(wq_g),
            "wk8": pack_w8(wk_g),
            "xbf": xbf,
            "wvT": wvp,
            "woT": wop,
            "cosC": cosC, "sinS": sinS,
        })
    return in_maps


def kernel(x, wq, wk, wv, wo, freqs_cos, freqs_sin, start_pos=0, _trace=False):
    x = np.asarray(x, dtype=np.float32)
    wq = np.asarray(wq, np.float32)
    wk = np.asarray(wk, np.float32)
    wv = np.asarray(wv, np.float32)
    wo = np.asarray(wo, np.float32)
    freqs_cos = np.asarray(freqs_cos, np.float32)
    freqs_sin = np.asarray(freqs_sin, np.float32)

    nc = _build()
    in_maps = _prep_inputs(x, wq, wk, wv, wo, freqs_cos, freqs_sin)
    try:
        res = run_bass_kernel_spmd(nc, in_maps, core_ids=list(range(8)),
                                   trace=_trace)
    except ModuleNotFoundError:
        res = run_bass_kernel_spmd(nc, in_maps, core_ids=list(range(8)),
                                   trace=False)
    out = np.zeros((BT, T, D), np.float32)
    for c in range(8):
        out[c // NG] += np.asarray(res.results[c]["out1"], np.float32)
        out[c // NG] += np.asarray(res.results[c]["out2"], np.float32)
    if _trace:
        kernel.last_results = res
    return out
